# revision 4
# baseline (speedup 1.0000x reference)
"""HardTripletMiningLoss on 8 TRN2 NeuronCores (Bass, raw-block SPMD).

Math: with emb = concat(anchor, positive, negative) [N,D], labels = ind[:,0],
pd(a,b) = ||e_a - e_b||^2, the loss is the mean over triplets (i,j,k) of
td = pd(i,j) - pd(j,k) + A restricted to
  same(i,j) & ~same(j,k) & td > 0 & i != 0.
Only (i,j) pairs with same labels (and i>=1) contribute — ~N^2/L of N^2 pairs.
Each such pair p=(i,j) needs, over k: sum/count of relu(td), where
  td(p,k) = 2*g(j,k) - n_k + (n_i - 2*u_p + A),  u_p = <e_i, e_j>,
and same-label k are masked by adding -BIG inside the matmul accumulation.

Per core: pairs become rows of [128, N] tiles; PE computes
  V'[p,k] = g(j_p,k) - n_k/2 - (BIG/2)*same(j_p,k)
via two accumulating matmuls (emb^T gathered by j, then a [1+L, .] aux matmul
whose row 0 carries -n_k/2 and rows 1..L carry the one-hot label mask), then
ACT does relu(2*V' + bias_p) with a fused row-sum and DVE does the fused
count (V' > -bias_p/2). Host sums the 8 cores' partials and forms the mean.
"""

import numpy as np
from contextlib import ExitStack

import concourse.bass as bass
import concourse.mybir as mybir
from concourse.bass_utils import run_bass_kernel_spmd

F32 = mybir.dt.float32
AF = mybir.ActivationFunctionType
OP = mybir.AluOpType

N_CORES = 8
A_MARGIN = 0.2
BIG_HALF = 1.0e6  # BIG = 2e6 dominates any |td| (~1e3) by >>1e3x
PAD_NEG = -1.0e9  # bias for padding pair rows -> relu 0, count 0
MAX_TILES = 4     # per-core pair tiles per launch (PSUM bank budget)

_programs: dict = {}  # (T, N, L) -> bass.Bass
LAST_RESULTS: list = []  # BassKernelResults of the launches in the last kernel() call


def _build_program(T: int, N: int, L: int) -> "bass.Bass":
    """One SPMD program: every core runs this with its own pair shard."""
    P = T * 128
    nc = bass.Bass()

    d_embt = nc.declare_dram_parameter("embt", [128, N], F32, isOutput=False)
    d_embjt = nc.declare_dram_parameter("embjt", [128, P], F32, isOutput=False)
    d_embit = nc.declare_dram_parameter("embit", [128, P], F32, isOutput=False)
    d_auxl = nc.declare_dram_parameter("auxl", [1 + L, P], F32, isOutput=False)
    d_auxrh = nc.declare_dram_parameter("auxrh", [L, N], F32, isOutput=False)
    d_padcol = nc.declare_dram_parameter("padcol", [128, T], F32, isOutput=False)
    d_out_s = nc.declare_dram_parameter("out_s", [128, T], F32, isOutput=True)
    d_out_c = nc.declare_dram_parameter("out_c", [128, T], F32, isOutput=True)

    with ExitStack() as ctx:
        sb = lambda name, shape: ctx.enter_context(nc.sbuf_tensor(name, shape, F32))
        ps = lambda name, shape: ctx.enter_context(nc.psum_tensor(name, shape, F32))

        embt_sb = sb("embt_sb", [128, N])
        embjt_sb = sb("embjt_sb", [128, P])
        embit_sb = sb("embit_sb", [128, P])
        auxl_sb = sb("auxl_sb", [1 + L, P])
        # row 0: -n_k/2 (device), rows 1..L: host mask
        auxr_sb = sb("auxr_sb", [1 + L, N])
        padcol_sb = sb("padcol_sb", [128, T])
        sq_t = sb("sq_t", [128, N])
        tmp_u = sb("tmp_u", [128, P])
        sqi = sb("sqi", [128, P])
        bias_row = sb("bias_row", [1, P])
        thresh_col = sb("thresh_col", [128, T])
        rs_col = sb("rs_col", [128, T])
        rc_col = sb("rc_col", [128, T])
        scratch_a = sb("scratch_a", [128, N])
        zeros_nt = sb("zeros_nt", [128, N])
        scratch_d = sb("scratch_d", [128, N])
        ones128 = sb("ones128", [128, 1])
        neg2_128 = sb("neg2_128", [128, 1])
        ones11 = sb("ones11", [1, 1])

        psum_n = ps("psum_n", [1, N])
        psum_u = ps("psum_u", [1, P])
        psum_bias = ps("psum_bias", [128, T])
        psumV = [ps(f"psumV{t}", [128, N]) for t in range(T)]

        with (
            nc.semaphore("dma_in") as dma_in,
            nc.semaphore("dma_out") as dma_out,
            nc.semaphore("v_pre") as v_pre,
            nc.semaphore("a_pre") as a_pre,
            nc.semaphore("pe_pre") as pe_pre,
            nc.semaphore("mm") as mm,
            nc.semaphore("dve_s") as dve_s,
            nc.Block() as block,
        ):

            @block.sync
            def _(sync):
                sync.dma_start(embt_sb[:], d_embt[:]).then_inc(dma_in, 16)
                sync.dma_start(embjt_sb[:], d_embjt[:]).then_inc(dma_in, 16)
                sync.dma_start(embit_sb[:], d_embit[:]).then_inc(dma_in, 16)
                sync.dma_start(auxl_sb[:], d_auxl[:]).then_inc(dma_in, 16)
                sync.dma_start(auxr_sb[1:1 + L, :], d_auxrh[:]).then_inc(dma_in, 16)
                sync.dma_start(padcol_sb[:], d_padcol[:]).then_inc(dma_in, 16)
                sync.wait_ge(dve_s, 2 * T)
                sync.dma_start(d_out_s[:], rs_col[:]).then_inc(dma_out, 16)
                sync.dma_start(d_out_c[:], rc_col[:]).then_inc(dma_out, 16)
                sync.wait_ge(dma_out, 32)

            @block.vector
            def _(vector):
                nc.vector.memset(ones128[:], 1.0).then_inc(v_pre, 1)   # 1
                nc.vector.memset(ones11[:], 1.0).then_inc(v_pre, 1)    # 2
                nc.vector.memset(neg2_128[:], -2.0).then_inc(v_pre, 1)  # 3
                nc.vector.memset(zeros_nt[:], 0.0).then_inc(v_pre, 1)  # 4
                vector.wait_ge(dma_in, 96)
                nc.vector.tensor_tensor(
                    sq_t[:], embt_sb[:], embt_sb[:], OP.mult
                ).then_inc(v_pre, 1)                                   # 5
                nc.vector.tensor_tensor(
                    tmp_u[:], embit_sb[:], embjt_sb[:], OP.mult
                ).then_inc(v_pre, 1)                                   # 6
                nc.vector.tensor_tensor(
                    sqi[:], embit_sb[:], embit_sb[:], OP.mult
                ).then_inc(v_pre, 1)                                   # 7
                vector.wait_ge(pe_pre, 2)
                # bias_row = -2*u + n_i accumulated in psum_u by PE
                nc.vector.tensor_copy(bias_row[:], psum_u[:]).then_inc(v_pre, 1)  # 8
                vector.wait_ge(pe_pre, 2 + T)
                # thresh = -(bias + pad)/2; host sends padcol pre-halved/negated
                nc.vector.scalar_tensor_tensor(
                    thresh_col[:], psum_bias[:], -0.5, padcol_sb[:], OP.mult, OP.add
                ).then_inc(v_pre, 1)                                   # 9
                vector.wait_ge(v_pre, 9)
                for t in range(T):
                    vector.wait_ge(mm, t + 1)
                    if t > 0:
                        vector.wait_ge(dve_s, 2 * t - 1)
                    # sum of relu((2V'+bias)/2) -> host multiplies by 2
                    nc.vector.scalar_tensor_tensor(
                        scratch_a[:], psumV[t][:], thresh_col[:, t:t + 1],
                        zeros_nt[:], OP.subtract, OP.max,
                        accum_out=rs_col[:, t:t + 1],
                    ).then_inc(dve_s, 1)
                    if t > 0:
                        vector.wait_ge(dve_s, 2 * t)
                    nc.vector.tensor_scalar(
                        scratch_d[:], psumV[t][:], thresh_col[:, t:t + 1], None,
                        OP.is_gt, OP.add, accum_out=rc_col[:, t:t + 1],
                    ).then_inc(dve_s, 1)

            @block.scalar
            def _(scalar):
                scalar.wait_ge(pe_pre, 1)
                nc.scalar.mul(auxr_sb[0:1, :], psum_n[0:1, :], -0.5).then_inc(a_pre, 1)

            @block.tensor
            def _(tensor):
                tensor.wait_ge(v_pre, 5)
                nc.tensor.matmul(
                    psum_n[:], ones128[:], sq_t[:], start=True, stop=True
                ).then_inc(pe_pre, 1)
                tensor.wait_ge(v_pre, 7)
                nc.tensor.matmul(
                    psum_u[:], neg2_128[:], tmp_u[:], start=True, stop=False
                )
                nc.tensor.matmul(
                    psum_u[:], ones128[:], sqi[:], start=False, stop=True
                ).then_inc(pe_pre, 1)
                tensor.wait_ge(v_pre, 8)
                for t in range(T):
                    # transpose bias_row chunk t -> psum_bias[:, t]
                    nc.tensor.matmul(
                        psum_bias[:, t:t + 1], bias_row[0:1, bass.ts(t, 128)],
                        ones11[:], start=True, stop=True,
                    ).then_inc(pe_pre, 1)
                tensor.wait_ge(dma_in, 96)
                tensor.wait_ge(a_pre, 1)
                for t in range(T):
                    nc.tensor.matmul(
                        psumV[t][:], embjt_sb[:, bass.ts(t, 128)], embt_sb[:],
                        start=True, stop=False,
                    )
                    nc.tensor.matmul(
                        psumV[t][:], auxl_sb[:, bass.ts(t, 128)], auxr_sb[:],
                        start=False, stop=True,
                    ).then_inc(mm, 1)

    return nc


def _get_program(T: int, N: int, L: int) -> "bass.Bass":
    key = (T, N, L)
    if key not in _programs:
        _programs[key] = _build_program(T, N, L)
    return _programs[key]


def _run_batch(emb, labels, sq_labels_masks, ii, jj, T):
    """Run one SPMD launch over <=8*T*128 pairs; returns (sum, count) f64."""
    N, D = emb.shape
    L, embt, auxrh = sq_labels_masks
    P = T * 128
    per = (len(ii) + N_CORES - 1) // N_CORES

    in_maps = []
    for c in range(N_CORES):
        si = ii[c * per:(c + 1) * per]
        sj = jj[c * per:(c + 1) * per]
        m = len(si)
        embjt = np.zeros((D, P), np.float32)
        embit = np.zeros((D, P), np.float32)
        auxl = np.zeros((1 + L, P), np.float32)
        flat_pad = np.full(P, -0.5 * PAD_NEG, np.float32)
        if m:
            embjt[:, :m] = emb[sj].T
            embit[:, :m] = emb[si].T
            auxl[0, :m] = 1.0
            auxl[1 + labels[sj], np.arange(m)] = 1.0
            flat_pad[:m] = -0.5 * A_MARGIN
        padcol = np.ascontiguousarray(flat_pad.reshape(T, 128).T)
        in_maps.append({
            "embt": embt,
            "embjt": embjt,
            "embit": embit,
            "auxl": auxl,
            "auxrh": auxrh,
            "padcol": padcol,
        })

    nc = _get_program(T, N, L)
    res = run_bass_kernel_spmd(nc, in_maps, list(range(N_CORES)))
    LAST_RESULTS.append(res)
    s = 0.0
    cnt = 0.0
    for c in range(N_CORES):
        s += 2.0 * float(res.results[c]["out_s"].astype(np.float64).sum())
        cnt += float(res.results[c]["out_c"].astype(np.float64).sum())
    return s, cnt


def kernel(anchor, positive, negative, ind):
    LAST_RESULTS.clear()
    anchor = np.asarray(anchor, dtype=np.float32)
    positive = np.asarray(positive, dtype=np.float32)
    negative = np.asarray(negative, dtype=np.float32)
    labels = np.asarray(ind).reshape(-1).astype(np.int64)

    emb = np.ascontiguousarray(np.concatenate([anchor, positive, negative], axis=0))
    N, D = emb.shape
    assert D == 128, f"kernel assumes D=128, got {D}"
    assert N == labels.shape[0]

    L = int(labels.max()) + 1 if labels.size else 1
    assert L <= 127, f"label ids must fit one-hot partitions, got {L}"

    # same-label (i, j) pairs, excluding the i=0 plane (keep[0] = False)
    same = labels[:, None] == labels[None, :]
    ii, jj = np.nonzero(same)
    sel = ii >= 1
    ii, jj = ii[sel].astype(np.int64), jj[sel].astype(np.int64)

    if len(ii) == 0:
        return np.float32(0.0)

    embt = np.ascontiguousarray(emb.T)
    auxrh = np.zeros((L, N), np.float32)
    auxrh[labels, np.arange(N)] = -BIG_HALF
    shared = (L, embt, auxrh)

    batch_cap = N_CORES * MAX_TILES * 128
    s_tot, c_tot = 0.0, 0.0
    for b0 in range(0, len(ii), batch_cap):
        bi, bj = ii[b0:b0 + batch_cap], jj[b0:b0 + batch_cap]
        per = (len(bi) + N_CORES - 1) // N_CORES
        T = max(1, (per + 127) // 128)
        s, c = _run_batch(emb, labels, shared, bi, bj, T)
        s_tot += s
        c_tot += c

    if c_tot > 0:
        return np.float32(s_tot / max(c_tot, 1.0))
    return np.float32(0.0)



# revision 5
# speedup vs baseline: 2.0238x; 2.0238x over previous
"""HardTripletMiningLoss on 8 TRN2 NeuronCores (Bass, raw-block SPMD).

Math: with emb = concat(anchor, positive, negative) [N,D], labels = ind[:,0],
pd(a,b) = ||e_a - e_b||^2, the loss is the mean over triplets (i,j,k) of
td = pd(i,j) - pd(j,k) + A restricted to
  same(i,j) & ~same(j,k) & td > 0 & i != 0.
Only (i,j) pairs with same labels (and i>=1) contribute — ~N^2/L of N^2 pairs.
Each such pair p=(i,j) needs, over k: sum/count of relu(td), where
  td(p,k) = 2*[g(j_p,k) - n_k/2 - (BIG/2)*same(j_p,k)] + (n_i - 2*u_p + A)
with g = emb gram, n = squared norms, u_p = <e_i, e_j>.

Per core: pairs become rows of [128, N] PSUM tiles V' via two accumulating
bf16 matmuls: emb^T gathered by j against emb^T, plus a [1+L, .] aux matmul
whose lhs rows are (all-ones, one-hot(label_j)) and rhs rows are
(bf16 residual of -n_k/2, bf16(-n_k/2) - (BIG/2)*onehot mask).  The bf16
rounding of -n_k/2 is corrected by the residual row, so the only bf16 error
is the 0.4% rounding of emb itself (~2e-5 on the final mean).
Host precomputes per-pair thresholds t_p = -(n_i - 2u_p + A)/2:
  sum_k relu(td)/2 = relu-row-sum of (V' - t_p)  -> scalar engine ACT
  count_k          = row-sum of (V' > t_p)       -> vector engine DVE
running in parallel on the two engines.  The tensor engine warms the HAM
clock gate with junk matmuls while input DMAs are in flight.  Host sums the
8 cores' [128, 2T] partials and forms the mean.
"""

import numpy as np
from contextlib import ExitStack

import ml_dtypes

import concourse.bass as bass
import concourse.mybir as mybir
from concourse.bass_utils import run_bass_kernel_spmd

F32 = mybir.dt.float32
BF16 = mybir.dt.bfloat16
AF = mybir.ActivationFunctionType
OP = mybir.AluOpType
NP_BF16 = ml_dtypes.bfloat16

N_CORES = 8
A_MARGIN = 0.2
BIG_HALF = 1.0e6   # BIG = 2e6 dominates any |td| (~1e3) by >>1e3x
PAD_THRESH = 0.5e9  # threshold for padding pair rows -> relu 0, count 0
MAX_TILES = 4      # per-core pair tiles per launch (PSUM bank budget)
NUM_WARM = 20      # junk matmuls to warm the PE clock gate during DMA wait

_programs: dict = {}  # (T, N, L) -> bass.Bass
LAST_RESULTS: list = []  # BassKernelResults of the launches in the last kernel() call


def _build_program(T: int, N: int, L: int) -> "bass.Bass":
    """One SPMD program: every core runs this with its own pair shard."""
    P = T * 128
    R = 1 + L  # aux contraction rows: residual row + one-hot label rows
    nc = bass.Bass()

    d_emb = nc.declare_dram_parameter("emb", [128, N + P], BF16, isOutput=False)
    d_aux = nc.declare_dram_parameter("aux", [R, P + N], BF16, isOutput=False)
    d_scal = nc.declare_dram_parameter("scal", [128, 2 * T], F32, isOutput=False)
    d_out = nc.declare_dram_parameter("out", [128, 2 * T], F32, isOutput=True)

    with ExitStack() as ctx:
        sb = lambda name, shape, dt: ctx.enter_context(nc.sbuf_tensor(name, shape, dt))
        ps = lambda name, shape: ctx.enter_context(nc.psum_tensor(name, shape, F32))

        emb_sb = sb("emb_sb", [128, N + P], BF16)   # [:, :N] emb^T, [:, N:] gathered j
        aux_sb = sb("aux_sb", [R, P + N], BF16)     # [:, :P] lhs, [:, P:] rhs
        scal_sb = sb("scal_sb", [128, 2 * T], F32)  # [:, :T] -thresh, [:, T:] +thresh
        out_sb = sb("out_sb", [128, 2 * T], F32)    # [:, :T] relu sums, [:, T:] counts
        act_junk = sb("act_junk", [128, N], BF16)
        dve_junk = sb("dve_junk", [128, N], BF16)
        warm_w = sb("warm_w", [128, 128], BF16)     # uninitialized junk, warmup only

        psum_warm = ps("psum_warm", [128, 128])
        psumV = [ps(f"psumV{t}", [128, N]) for t in range(T)]

        with (
            nc.semaphore("dma_in") as dma_in,
            nc.semaphore("dma_out") as dma_out,
            nc.semaphore("mm") as mm,
            nc.semaphore("act_s") as act_s,
            nc.semaphore("dve_s") as dve_s,
            nc.Block() as block,
        ):

            @block.sync
            def _(sync):
                sync.dma_start(emb_sb[:], d_emb[:]).then_inc(dma_in, 16)
                sync.wait_ge(act_s, T)
                sync.wait_ge(dve_s, T)
                sync.dma_start(d_out[:], out_sb[:]).then_inc(dma_out, 16)
                sync.wait_ge(dma_out, 16)

            @block.scalar
            def _(scalar):
                scalar.dma_start(aux_sb[:], d_aux[:]).then_inc(dma_in, 16)
                scalar.dma_start(scal_sb[:], d_scal[:]).then_inc(dma_in, 16)
                for t in range(T):
                    scalar.wait_ge(mm, t + 1)
                    # relu(V' - thresh) row-summed into out_sb[:, t]
                    nc.scalar.activation(
                        act_junk[:], psumV[t][:], AF.Relu,
                        bias=scal_sb[:, t:t + 1],
                        accum_out=out_sb[:, t:t + 1],
                    ).then_inc(act_s, 1)

            @block.tensor
            def _(tensor):
                for _ in range(NUM_WARM):
                    nc.tensor.matmul(
                        psum_warm[:], warm_w[:], warm_w[:], start=True, stop=True
                    )
                tensor.wait_ge(dma_in, 48)
                for t in range(T):
                    nc.tensor.matmul(
                        psumV[t][:], emb_sb[:, N + t * 128:N + (t + 1) * 128],
                        emb_sb[:, 0:N], start=True, stop=False,
                    )
                    nc.tensor.matmul(
                        psumV[t][:], aux_sb[:, t * 128:(t + 1) * 128],
                        aux_sb[:, P:P + N], start=False, stop=True,
                    ).then_inc(mm, 1)

            @block.vector
            def _(vector):
                for t in range(T):
                    vector.wait_ge(mm, t + 1)
                    # count of V' > thresh row-summed into out_sb[:, T + t]
                    nc.vector.tensor_scalar(
                        dve_junk[:], psumV[t][:], scal_sb[:, T + t:T + t + 1], None,
                        OP.is_gt, OP.add, accum_out=out_sb[:, T + t:T + t + 1],
                    ).then_inc(dve_s, 1)

    return nc


def _get_program(T: int, N: int, L: int) -> "bass.Bass":
    key = (T, N, L)
    if key not in _programs:
        _programs[key] = _build_program(T, N, L)
    return _programs[key]


def _run_batch(emb_bf, labels, shared, ii, jj, thresh, T):
    """Run one SPMD launch over <=8*T*128 pairs; returns (sum, count) f64."""
    D, N = emb_bf.shape[0], emb_bf.shape[1]
    L, auxr = shared
    R = 1 + L
    P = T * 128
    per = (len(ii) + N_CORES - 1) // N_CORES

    in_maps = []
    for c in range(N_CORES):
        sj = jj[c * per:(c + 1) * per]
        m = len(sj)
        emb_blob = np.zeros((128, N + P), NP_BF16)
        emb_blob[:, :N] = emb_bf
        aux_blob = np.zeros((R, P + N), NP_BF16)
        aux_blob[:, P:] = auxr
        scal = np.empty((2 * T, 128), np.float32)  # [2T,128] then transpose
        scal[:T] = -PAD_THRESH
        scal[T:] = PAD_THRESH
        if m:
            emb_blob[:, N:N + m] = emb_bf[:, sj]
            aux_blob[0, :m] = 1.0
            aux_blob[1 + labels[sj], np.arange(m)] = 1.0
            th = thresh[c * per:c * per + m]
            flat = scal.reshape(2, T * 128)
            flat[0, :m] = -th
            flat[1, :m] = th
        in_maps.append({
            "emb": emb_blob,
            "aux": aux_blob,
            "scal": np.ascontiguousarray(scal.reshape(2, T, 128).transpose(2, 0, 1)
                                         .reshape(128, 2 * T)),
        })

    nc = _get_program(T, N, L)
    res = run_bass_kernel_spmd(nc, in_maps, list(range(N_CORES)))
    LAST_RESULTS.append(res)
    s = 0.0
    cnt = 0.0
    for c in range(N_CORES):
        out = res.results[c]["out"].astype(np.float64)
        s += 2.0 * float(out[:, :T].sum())
        cnt += float(out[:, T:].sum())
    return s, cnt


def kernel(anchor, positive, negative, ind):
    LAST_RESULTS.clear()
    anchor = np.asarray(anchor, dtype=np.float32)
    positive = np.asarray(positive, dtype=np.float32)
    negative = np.asarray(negative, dtype=np.float32)
    labels = np.asarray(ind).reshape(-1).astype(np.int64)

    emb = np.ascontiguousarray(np.concatenate([anchor, positive, negative], axis=0))
    N, D = emb.shape
    assert D == 128, f"kernel assumes D=128, got {D}"
    assert N == labels.shape[0]

    L = int(labels.max()) + 1 if labels.size else 1
    assert L <= 127, f"label ids must fit one-hot partitions, got {L}"

    # same-label (i, j) pairs, excluding the i=0 plane (keep[0] = False)
    same = labels[:, None] == labels[None, :]
    ii, jj = np.nonzero(same)
    sel = ii >= 1
    ii, jj = ii[sel].astype(np.int64), jj[sel].astype(np.int64)

    if len(ii) == 0:
        return np.float32(0.0)

    n = (emb * emb).sum(axis=1, dtype=np.float32)          # squared norms [N]
    u = (emb[ii] * emb[jj]).sum(axis=1, dtype=np.float32)  # <e_i, e_j> per pair
    # thresh_p = -(n_i - 2 u_p + A)/2; hard iff V' > thresh_p
    thresh = (-0.5 * (n[ii] - 2.0 * u + A_MARGIN)).astype(np.float32)

    emb_bf = np.ascontiguousarray(emb.T).astype(NP_BF16)   # [D, N]

    # aux rhs [1+L, N]: row 0 = residual of bf16(-n/2); rows 1+l = bf16(-n/2)
    # with -BIG/2 added where labels==l (bf16 swallows -n/2 there; masked anyway)
    mk = (-0.5 * n).astype(NP_BF16)
    rk = (-0.5 * n - mk.astype(np.float32)).astype(NP_BF16)
    auxr = np.zeros((1 + L, N), NP_BF16)
    auxr[0] = rk
    auxr[1:] = mk[None, :]
    lab_cols = np.arange(N)
    auxr[1 + labels, lab_cols] = (mk.astype(np.float32)[lab_cols] - BIG_HALF
                                  ).astype(NP_BF16)
    shared = (L, auxr)

    batch_cap = N_CORES * MAX_TILES * 128
    s_tot, c_tot = 0.0, 0.0
    for b0 in range(0, len(ii), batch_cap):
        bi, bj = ii[b0:b0 + batch_cap], jj[b0:b0 + batch_cap]
        bt = thresh[b0:b0 + batch_cap]
        per = (len(bi) + N_CORES - 1) // N_CORES
        T = max(1, (per + 127) // 128)
        s, c = _run_batch(emb_bf, labels, shared, bi, bj, bt, T)
        s_tot += s
        c_tot += c

    if c_tot > 0:
        return np.float32(s_tot / max(c_tot, 1.0))
    return np.float32(0.0)
